# revision 10
# baseline (speedup 1.0000x reference)
"""3-layer GAT + linear head on 8 TRN2 NeuronCores (Bass/Tile).

Sharding (follows the problem hint):
  - Nodes split into 8 contiguous blocks of 6250; core k owns block k and
    every edge whose destination lies in its block (sorted by destination).
  - Per layer: each core computes H = X @ W.T (+ attention projections) for
    its own nodes, AllGathers the rows into a replicated node table, then
    processes its edges per 128-destination tile:
      * dma_gather of source rows (edge slot e=ch*128+p -> partition p,
        chunk ch),
      * per-edge attention w = exp(leakyrelu(a_s[src] + a_d[dst])),
      * segment softmax + weighted sum via a one-hot matmul on TensorE:
        S01[e,d] = (dst_loc[e]==d); out[d,:] += S01.T @ (w*G); s[d] += S01.T @ w
      * out/s, +bias, ELU; PE-transpose the tile for the next layer matmul.
  - a_d[dst] broadcast edge-wise = PE transpose of S01 + tiny matmul against
    the tile's a_d column (destinations are always core-local).
  - int16 gather indices: node table split in two 25000-row halves, each
    dst-tile's edges pre-split by source half (groups A/B).

Self-contained; hardcodes shapes for N=50000, E=800000, D_IN=128, HID=64,
HEADS=8, D_OUT=10.
"""
import os
import numpy as np

import concourse.bass as bass
import concourse.mybir as mybir
import concourse.tile as tile
from concourse import bacc
from concourse.bass_utils import run_bass_kernel_spmd
from concourse.masks import make_identity

N = 50000
E = 800000
NCORES = 8
VP = N // NCORES          # 6250 nodes per core
P = 128
NT = (VP + P - 1) // P    # 49 dst tiles per core (last has 106 rows)
NTP = NT * P              # 6272
HALF = N // 2             # 25000
D_IN = 128
HID = 64
HEADS = 8
D_OUT = 10
R1 = 576                  # layer-1 row: h(512) | a_s(8) | pad(56) -> 2304B
R2 = HID                  # layer-2/3 row: h(64) -> 256B

f32 = mybir.dt.float32
i16 = mybir.dt.int16
AT = mybir.AluOpType
AF = mybir.ActivationFunctionType


def _prep_edges(edge_index):
    src = np.concatenate([np.asarray(edge_index[0]), np.arange(N)]).astype(np.int64)
    dst = np.concatenate([np.asarray(edge_index[1]), np.arange(N)]).astype(np.int64)

    per_core = []
    maxch = [0, 0]
    for k in range(NCORES):
        m = (dst >= k * VP) & (dst < (k + 1) * VP)
        s_k = src[m]
        dloc = dst[m] - k * VP
        t_k = dloc // P
        w_k = dloc % P
        tiles = []
        for t in range(NT):
            sel = t_k == t
            ss, ww = s_k[sel], w_k[sel]
            groups = []
            for g in range(2):
                gm = (ss < HALF) if g == 0 else (ss >= HALF)
                li = (ss[gm] - g * HALF).astype(np.int64)
                groups.append((li, ww[gm].astype(np.int64)))
                maxch[g] = max(maxch[g], (len(li) + P - 1) // P)
            tiles.append(groups)
        per_core.append(tiles)

    chs = [max(c, 1) for c in maxch]
    idx_arrs, dst_arrs = [], []
    for g in range(2):
        ch = chs[g]
        ia, da = [], []
        for k in range(NCORES):
            A = np.zeros((NT, P, ch * 8), np.int16)
            D = np.full((NT, P, ch), -1.0, np.float32)
            for t in range(NT):
                li, ww = per_core[k][t][g]
                n = len(li)
                ii = np.arange(n)
                wrap = np.zeros((16, ch * 8), np.int16)
                wrap[ii % 16, ii // 16] = li.astype(np.int16)
                A[t] = np.tile(wrap, (8, 1))
                D[t, ii % P, ii // P] = ww
            ia.append(A)
            da.append(D)
        idx_arrs.append(ia)
        dst_arrs.append(da)
    return chs, idx_arrs, dst_arrs


def _edge_phase(nc, tc, layer, CHs, idx_ins, dst_ins, hfull, Rrow, heads,
                ad_sb, vrep, iota, ident, brep, hT_d, rows_of):
    HC = 512 if layer == 1 else HID
    CHa, CHb = CHs
    elite = int(os.environ.get("GAT_ELITE", "5"))
    with tc.tile_pool(name=f"e{layer}", bufs=2) as ep, \
         tc.tile_pool(name=f"e{layer}w", bufs=4) as wp, \
         tc.tile_pool(name=f"e{layer}s", bufs=max(CHa, CHb) + 2) as sp, \
         tc.tile_pool(name=f"e{layer}p1", bufs=2, space="PSUM") as pp, \
         tc.tile_pool(name=f"e{layer}p2", bufs=2, space="PSUM") as p2:
        for t in range(NT):
            if elite >= 4:
                outu = p2.tile([P, HC], f32, space="PSUM", tag="outu")
                ssum = p2.tile([P, heads], f32, space="PSUM", tag="ssum")
            for g in range(2):
                CH = CHs[g]
                NIDX = CH * P
                idxt = wp.tile([P, CH * 8], i16, tag="idx")
                nc.sync.dma_start(out=idxt[:], in_=idx_ins[g][t])
                dstt = wp.tile([P, CH], f32, tag="dst")
                nc.sync.dma_start(out=dstt[:], in_=dst_ins[g][t])
                G = ep.tile([P, CH, Rrow], f32, tag="G")
                half = hfull[g * HALF:(g + 1) * HALF, :]
                nc.gpsimd.dma_gather(G[:], half, idxt[:], NIDX, NIDX, Rrow,
                                     single_packet=False)
                est = wp.tile([P, CH, heads], f32, tag="est")
                s01s = []
                if elite >= 2:
                  for ch in range(CH):
                    s01 = sp.tile([P, P], f32, tag="s01")
                    nc.vector.tensor_scalar(
                        out=s01[:], in0=iota[:], scalar1=dstt[:, ch:ch + 1],
                        scalar2=None, op0=AT.is_equal)
                    s01t_ps = pp.tile([P, P], f32, space="PSUM", tag="s01t")
                    nc.tensor.transpose(out=s01t_ps[:], in_=s01[:],
                                        identity=ident[:])
                    s01t = wp.tile([P, P], f32, tag="s01t_sb")
                    nc.scalar.activation(s01t[:], s01t_ps[:], AF.Copy)
                    adg_ps = pp.tile([P, heads], f32, space="PSUM", tag="adg")
                    nc.tensor.matmul(
                        adg_ps[:], lhsT=s01t[:],
                        rhs=ad_sb[:, t * heads:(t + 1) * heads],
                        start=True, stop=True)
                    if elite >= 3:
                        if layer == 1:
                            nc.vector.tensor_tensor(
                                out=est[:, ch, :], in0=G[:, ch, 512:520],
                                in1=adg_ps[:], op=AT.add)
                        else:
                            scr = wp.tile([P, Rrow], f32, tag="scr")
                            nc.vector.scalar_tensor_tensor(
                                out=scr[:], in0=G[:, ch, :], scalar=1.0,
                                in1=vrep[:], op0=AT.mult, op1=AT.mult,
                                accum_out=est[:, ch, 0:1])
                            nc.vector.tensor_tensor(
                                out=est[:, ch, 0:1], in0=est[:, ch, 0:1],
                                in1=adg_ps[:], op=AT.add)
                    s01s.append(s01)
                if elite >= 3:
                    # batched leaky-relu + exp over this group's edge weights
                    ef = est[:].rearrange("p a b -> p (a b)")
                    nc.vector.scalar_tensor_tensor(
                        out=ef, in0=ef, scalar=0.2, in1=ef,
                        op0=AT.mult, op1=AT.max)
                    nc.scalar.activation(ef, ef, AF.Exp)
                if elite < 4:
                    continue
                # weight G rows, aggregate
                for ch in range(CH):
                    if heads == 8:
                        gv = G[:, ch, 0:512].rearrange("p (h c) -> p h c", h=8)
                        nc.vector.tensor_tensor(
                            out=gv, in0=gv,
                            in1=est[:, ch, :].to_broadcast([P, 8, 64]),
                            op=AT.mult)
                    else:
                        nc.vector.tensor_scalar(
                            out=G[:, ch, :], in0=G[:, ch, :],
                            scalar1=est[:, ch, 0:1], scalar2=None,
                            op0=AT.mult)
                    first = (g == 0 and ch == 0)
                    last = (g == 1 and ch == CH - 1)
                    nc.tensor.matmul(outu[:], lhsT=s01s[ch][:],
                                     rhs=G[:, ch, 0:HC],
                                     start=first, stop=last,
                                     skip_group_check=True)
                    nc.tensor.matmul(ssum[:], lhsT=s01s[ch][:],
                                     rhs=est[:, ch, :],
                                     start=first, stop=last,
                                     skip_group_check=True)
            if elite < 5:
                continue
            # epilogue: normalize, bias, ELU
            rec = wp.tile([P, heads], f32, tag="rec")
            nc.vector.reciprocal(rec[:], ssum[:])
            ho = ep.tile([P, HC], f32, tag="ho")
            if heads == 8:
                hv = ho[:].rearrange("p (h c) -> p h c", h=8)
                ov = outu[:].rearrange("p (h c) -> p h c", h=8)
                nc.vector.tensor_tensor(
                    out=hv, in0=ov, in1=rec[:].to_broadcast([P, 8, 64]),
                    op=AT.mult)
            else:
                nc.vector.tensor_scalar(
                    out=ho[:], in0=outu[:], scalar1=rec[:, 0:1],
                    scalar2=None, op0=AT.mult)
            nc.vector.tensor_tensor(out=ho[:], in0=ho[:], in1=brep[:],
                                    op=AT.add)
            el = ep.tile([P, HC], f32, tag="el")
            nc.vector.tensor_scalar(out=el[:], in0=ho[:], scalar1=0.0,
                                    scalar2=None, op0=AT.min)
            nc.scalar.activation(el[:], el[:], AF.Exp)
            nc.vector.scalar_tensor_tensor(
                out=ho[:], in0=ho[:], scalar=0.0, in1=el[:],
                op0=AT.max, op1=AT.add)
            nc.scalar.activation(ho[:], ho[:], AF.Copy, bias=-1.0)
            # transpose for next layer's matmul (lhsT layout)
            nblk = HC // P if HC % P == 0 else 1
            if HC == 512:
                for cb in range(4):
                    tp_ps = pp.tile([P, P], f32, space="PSUM", tag="s01t")
                    nc.tensor.transpose(out=tp_ps[:],
                                        in_=ho[:, cb * P:(cb + 1) * P],
                                        identity=ident[:])
                    tsb = wp.tile([P, P], f32, tag="tsb")
                    nc.vector.tensor_copy(tsb[:], tp_ps[:])
                    nc.sync.dma_start(
                        out=hT_d[cb * P:(cb + 1) * P, t * P:(t + 1) * P],
                        in_=tsb[:])
            else:
                tp_ps = pp.tile([P, P], f32, space="PSUM", tag="s01t")
                nc.tensor.transpose(out=tp_ps[:HID, :], in_=ho[:],
                                    identity=ident[:])
                tsb = wp.tile([HID, P], f32, tag="tsb64")
                nc.vector.tensor_copy(tsb[:], tp_ps[:HID, :])
                nc.sync.dma_start(out=hT_d[:, t * P:(t + 1) * P], in_=tsb[:])


PHASE_ORDER = ["m1", "ag1", "e1", "m2", "ag2", "e2", "m3", "ag3", "e3", "full"]


def _build_program(CHa, CHb):
    stop = os.environ.get("GAT_STOP", "full")
    lvl = PHASE_ORDER.index(stop) + 1
    nc = bacc.Bacc("TRN2", target_bir_lowering=False, debug=False,
                   enable_asserts=False, num_devices=NCORES)

    xT_in = nc.dram_tensor("xT", [P, NTP], f32, kind="ExternalInput")
    idxA_in = nc.dram_tensor("idxA", [NT, P, CHa * 8], i16, kind="ExternalInput")
    idxB_in = nc.dram_tensor("idxB", [NT, P, CHb * 8], i16, kind="ExternalInput")
    dstA_in = nc.dram_tensor("dstA", [NT, P, CHa], f32, kind="ExternalInput")
    dstB_in = nc.dram_tensor("dstB", [NT, P, CHb], f32, kind="ExternalInput")
    W1T_in = nc.dram_tensor("W1T", [D_IN, 512], f32, kind="ExternalInput")
    M1s_in = nc.dram_tensor("M1s", [D_IN, 8], f32, kind="ExternalInput")
    M1d_in = nc.dram_tensor("M1d", [D_IN, 8], f32, kind="ExternalInput")
    W2T_in = nc.dram_tensor("W2T", [512, HID], f32, kind="ExternalInput")
    M2d_in = nc.dram_tensor("M2d", [512, 1], f32, kind="ExternalInput")
    W3T_in = nc.dram_tensor("W3T", [HID, HID], f32, kind="ExternalInput")
    M3d_in = nc.dram_tensor("M3d", [HID, 1], f32, kind="ExternalInput")
    WcT_in = nc.dram_tensor("WcT", [HID, D_OUT], f32, kind="ExternalInput")
    b1r_in = nc.dram_tensor("b1r", [P, 512], f32, kind="ExternalInput")
    b2r_in = nc.dram_tensor("b2r", [P, HID], f32, kind="ExternalInput")
    b3r_in = nc.dram_tensor("b3r", [P, HID], f32, kind="ExternalInput")
    bcr_in = nc.dram_tensor("bcr", [P, D_OUT], f32, kind="ExternalInput")
    v2r_in = nc.dram_tensor("v2r", [P, HID], f32, kind="ExternalInput")
    v3r_in = nc.dram_tensor("v3r", [P, HID], f32, kind="ExternalInput")

    out_d = nc.dram_tensor("out", [NTP, D_OUT], f32, kind="ExternalOutput")

    hcat1_loc = nc.dram_tensor("hcat1_loc", [VP, R1], f32, kind="Internal")
    hcat1_full = nc.dram_tensor("hcat1_full", [N, R1], f32, kind="Internal",
                                addr_space="Shared")
    hcat2_loc = nc.dram_tensor("hcat2_loc", [VP, R2], f32, kind="Internal")
    hcat2_full = nc.dram_tensor("hcat2_full", [N, R2], f32, kind="Internal",
                                addr_space="Shared")
    hcat3_loc = nc.dram_tensor("hcat3_loc", [VP, R2], f32, kind="Internal")
    hcat3_full = nc.dram_tensor("hcat3_full", [N, R2], f32, kind="Internal",
                                addr_space="Shared")
    h1T_d = nc.dram_tensor("h1T_d", [512, NTP], f32, kind="Internal")
    h2T_d = nc.dram_tensor("h2T_d", [HID, NTP], f32, kind="Internal")
    h3T_d = nc.dram_tensor("h3T_d", [HID, NTP], f32, kind="Internal")

    def rows_of(t):
        return P if t < NT - 1 else VP - (NT - 1) * P

    rg = [list(range(NCORES))]

    with tile.TileContext(nc) as tc:
        with tc.tile_pool(name="const", bufs=1) as cs:
            ident = cs.tile([P, P], f32)
            make_identity(nc, ident[:])
            iota = cs.tile([P, P], f32)
            nc.gpsimd.iota(iota[:], pattern=[[1, P]], base=0,
                           channel_multiplier=0,
                           allow_small_or_imprecise_dtypes=True)

            def c_load(name, shape, src):
                tl = cs.tile(shape, f32, tag=name)
                nc.sync.dma_start(out=tl[:], in_=src)
                return tl

            W1T = c_load("W1T", [D_IN, 512], W1T_in[:])
            M1s = c_load("M1s", [D_IN, 8], M1s_in[:])
            M1d = c_load("M1d", [D_IN, 8], M1d_in[:])
            W2Tc = cs.tile([P, 4 * HID], f32)
            M2dc = cs.tile([P, 4], f32)
            for cb in range(4):
                nc.sync.dma_start(out=W2Tc[:, cb * HID:(cb + 1) * HID],
                                  in_=W2T_in[cb * P:(cb + 1) * P, :])
                nc.sync.dma_start(out=M2dc[:, cb:cb + 1],
                                  in_=M2d_in[cb * P:(cb + 1) * P, :])
            W3T = c_load("W3T", [HID, HID], W3T_in[:])
            M3d = c_load("M3d", [HID, 1], M3d_in[:])
            WcT = c_load("WcT", [HID, D_OUT], WcT_in[:])
            b1r = c_load("b1r", [P, 512], b1r_in[:])
            b2r = c_load("b2r", [P, HID], b2r_in[:])
            b3r = c_load("b3r", [P, HID], b3r_in[:])
            bcr = c_load("bcr", [P, D_OUT], bcr_in[:])
            v2r = c_load("v2r", [P, HID], v2r_in[:])
            v3r = c_load("v3r", [P, HID], v3r_in[:])
            ad1 = cs.tile([P, NT * 8], f32)
            ad2 = cs.tile([P, NT], f32)
            ad3 = cs.tile([P, NT], f32)

            # ---- M1 ----
            if lvl >= 1:
             with tc.tile_pool(name="m1", bufs=3) as mp, \
                 tc.tile_pool(name="m1p", bufs=2, space="PSUM") as mpp:
                for t in range(NT):
                    xt = mp.tile([P, P], f32, tag="xt")
                    nc.sync.dma_start(out=xt[:],
                                      in_=xT_in[:, t * P:(t + 1) * P])
                    h_ps = mpp.tile([P, 512], f32, space="PSUM", tag="h")
                    nc.tensor.matmul(h_ps[:], lhsT=xt[:], rhs=W1T[:],
                                     start=True, stop=True)
                    aa_ps = mpp.tile([P, 16], f32, space="PSUM", tag="aa")
                    nc.tensor.matmul(aa_ps[:, 0:8], lhsT=xt[:], rhs=M1s[:],
                                     start=True, stop=True)
                    nc.tensor.matmul(aa_ps[:, 8:16], lhsT=xt[:], rhs=M1d[:],
                                     start=True, stop=True)
                    hc = mp.tile([P, R1], f32, tag="hc")
                    nc.vector.tensor_copy(hc[:, 0:512], h_ps[:])
                    nc.scalar.activation(hc[:, 512:520], aa_ps[:, 0:8],
                                         AF.Copy)
                    nc.scalar.activation(ad1[:, t * 8:(t + 1) * 8],
                                         aa_ps[:, 8:16], AF.Copy)
                    r = rows_of(t)
                    nc.sync.dma_start(out=hcat1_loc[t * P:t * P + r, :],
                                      in_=hc[:r, :])
            if lvl >= 2:
             nc.gpsimd.collective_compute(
                "AllGather", AT.bypass, replica_groups=rg,
                ins=[hcat1_loc[:]], outs=[hcat1_full[:]])

            if lvl >= 3:
             _edge_phase(nc, tc, 1, (CHa, CHb), (idxA_in, idxB_in),
                        (dstA_in, dstB_in), hcat1_full, R1, 8, ad1, None,
                        iota, ident, b1r, h1T_d, rows_of)

            # ---- M2 ----
            if lvl >= 4:
             with tc.tile_pool(name="m2", bufs=3) as mp, \
                 tc.tile_pool(name="m2p", bufs=2, space="PSUM") as mpp:
                for t in range(NT):
                    h2_ps = mpp.tile([P, HID], f32, space="PSUM", tag="h")
                    ad_ps = mpp.tile([P, 1], f32, space="PSUM", tag="ad")
                    for cb in range(4):
                        lt = mp.tile([P, P], f32, tag="lt")
                        nc.sync.dma_start(
                            out=lt[:],
                            in_=h1T_d[cb * P:(cb + 1) * P, t * P:(t + 1) * P])
                        nc.tensor.matmul(
                            h2_ps[:], lhsT=lt[:],
                            rhs=W2Tc[:, cb * HID:(cb + 1) * HID],
                            start=(cb == 0), stop=(cb == 3))
                        nc.tensor.matmul(ad_ps[:], lhsT=lt[:],
                                         rhs=M2dc[:, cb:cb + 1],
                                         start=(cb == 0), stop=(cb == 3))
                    hc = mp.tile([P, R2], f32, tag="hc")
                    nc.vector.tensor_copy(hc[:], h2_ps[:])
                    nc.scalar.activation(ad2[:, t:t + 1], ad_ps[:], AF.Copy)
                    r = rows_of(t)
                    nc.sync.dma_start(out=hcat2_loc[t * P:t * P + r, :],
                                      in_=hc[:r, :])
            if lvl >= 5:
             nc.gpsimd.collective_compute(
                "AllGather", AT.bypass, replica_groups=rg,
                ins=[hcat2_loc[:]], outs=[hcat2_full[:]])

            if lvl >= 6:
             _edge_phase(nc, tc, 2, (CHa, CHb), (idxA_in, idxB_in),
                        (dstA_in, dstB_in), hcat2_full, R2, 1, ad2, v2r,
                        iota, ident, b2r, h2T_d, rows_of)

            # ---- M3 ----
            if lvl >= 7:
             with tc.tile_pool(name="m3", bufs=3) as mp, \
                 tc.tile_pool(name="m3p", bufs=2, space="PSUM") as mpp:
                for t in range(NT):
                    lt = mp.tile([HID, P], f32, tag="lt")
                    nc.sync.dma_start(out=lt[:],
                                      in_=h2T_d[:, t * P:(t + 1) * P])
                    h3_ps = mpp.tile([P, HID], f32, space="PSUM", tag="h")
                    nc.tensor.matmul(h3_ps[:], lhsT=lt[:], rhs=W3T[:],
                                     start=True, stop=True)
                    ad_ps = mpp.tile([P, 1], f32, space="PSUM", tag="ad")
                    nc.tensor.matmul(ad_ps[:], lhsT=lt[:], rhs=M3d[:],
                                     start=True, stop=True)
                    hc = mp.tile([P, R2], f32, tag="hc")
                    nc.vector.tensor_copy(hc[:], h3_ps[:])
                    nc.scalar.activation(ad3[:, t:t + 1], ad_ps[:], AF.Copy)
                    r = rows_of(t)
                    nc.sync.dma_start(out=hcat3_loc[t * P:t * P + r, :],
                                      in_=hc[:r, :])
            if lvl >= 8:
             nc.gpsimd.collective_compute(
                "AllGather", AT.bypass, replica_groups=rg,
                ins=[hcat3_loc[:]], outs=[hcat3_full[:]])

            if lvl >= 9:
             _edge_phase(nc, tc, 3, (CHa, CHb), (idxA_in, idxB_in),
                        (dstA_in, dstB_in), hcat3_full, R2, 1, ad3, v3r,
                        iota, ident, b3r, h3T_d, rows_of)

            # ---- final linear ----
            if lvl >= 10:
             with tc.tile_pool(name="fin", bufs=3) as mp, \
                 tc.tile_pool(name="finp", bufs=2, space="PSUM") as mpp:
                for t in range(NT):
                    lt = mp.tile([HID, P], f32, tag="lt")
                    nc.sync.dma_start(out=lt[:],
                                      in_=h3T_d[:, t * P:(t + 1) * P])
                    o_ps = mpp.tile([P, D_OUT], f32, space="PSUM", tag="o")
                    nc.tensor.matmul(o_ps[:], lhsT=lt[:], rhs=WcT[:],
                                     start=True, stop=True)
                    ob = mp.tile([P, D_OUT], f32, tag="ob")
                    nc.vector.tensor_tensor(out=ob[:], in0=o_ps[:],
                                            in1=bcr[:], op=AT.add)
                    r = rows_of(t)
                    nc.sync.dma_start(out=out_d[t * P:t * P + r, :],
                                      in_=ob[:r, :])

    nc.compile()
    return nc


def prepare(**inputs):
    """Host preprocessing + program build; returns (nc, in_maps)."""
    x = np.asarray(inputs["x"], np.float32)
    edge_index = np.asarray(inputs["edge_index"])
    W1 = np.asarray(inputs["W1"], np.float32)
    a1_src = np.asarray(inputs["a1_src"], np.float32)
    a1_dst = np.asarray(inputs["a1_dst"], np.float32)
    b1 = np.asarray(inputs["b1"], np.float32)
    W2 = np.asarray(inputs["W2"], np.float32)
    a2_src = np.asarray(inputs["a2_src"], np.float32)
    a2_dst = np.asarray(inputs["a2_dst"], np.float32)
    b2 = np.asarray(inputs["b2"], np.float32)
    W3 = np.asarray(inputs["W3"], np.float32)
    a3_src = np.asarray(inputs["a3_src"], np.float32)
    a3_dst = np.asarray(inputs["a3_dst"], np.float32)
    b3 = np.asarray(inputs["b3"], np.float32)
    Wc = np.asarray(inputs["Wc"], np.float32)
    bc = np.asarray(inputs["bc"], np.float32)

    (CHa, CHb), idx_arrs, dst_arrs = _prep_edges(edge_index)

    # weight preprocessing
    W1h = W1.reshape(HEADS, HID, D_IN)
    M1s = np.einsum("hci,hc->ih", W1h, a1_src).astype(np.float32)  # [128, 8]
    M1d = np.einsum("hci,hc->ih", W1h, a1_dst).astype(np.float32)
    M2d = (W2.T @ a2_dst[0]).reshape(512, 1).astype(np.float32)
    M3d = (W3.T @ a3_dst[0]).reshape(HID, 1).astype(np.float32)

    common = {
        "W1T": np.ascontiguousarray(W1.T),
        "M1s": M1s, "M1d": M1d,
        "W2T": np.ascontiguousarray(W2.T),
        "M2d": M2d,
        "W3T": np.ascontiguousarray(W3.T),
        "M3d": M3d,
        "WcT": np.ascontiguousarray(Wc.T),
        "b1r": np.tile(b1, (P, 1)),
        "b2r": np.tile(b2, (P, 1)),
        "b3r": np.tile(b3, (P, 1)),
        "bcr": np.tile(bc, (P, 1)),
        "v2r": np.tile(a2_src[0], (P, 1)),
        "v3r": np.tile(a3_src[0], (P, 1)),
    }

    in_maps = []
    for k in range(NCORES):
        xk = x[k * VP:(k + 1) * VP]
        xT = np.zeros((P, NTP), np.float32)
        xT[:, :VP] = xk.T
        m = dict(common)
        m["xT"] = xT
        m["idxA"] = idx_arrs[0][k]
        m["idxB"] = idx_arrs[1][k]
        m["dstA"] = dst_arrs[0][k]
        m["dstB"] = dst_arrs[1][k]
        in_maps.append(m)

    nc = _build_program(CHa, CHb)
    return nc, in_maps


def kernel(**inputs):
    nc, in_maps = prepare(**inputs)
    r = run_bass_kernel_spmd(nc, in_maps, core_ids=list(range(NCORES)))
    out = np.concatenate([r.results[k]["out"][:VP] for k in range(NCORES)], 0)
    return out.astype(np.float32)
